# revision 6
# baseline (speedup 1.0000x reference)
"""Trainium2 Bass kernel for a transformer decoder layer (self-attn + cross-attn + FFN).

Sharding: 8 cores = (batch b in {0,1}) x (query-row quarter r in {0..3}).
Each core computes, for its 512 query rows of its batch:
  - q1/k1/v1 projections (k1/v1 over the full 2048 rows of its batch),
  - masked softmax attention weights aw1 (written transposed, bf16),
  - attn1 -> LN1 -> out1, cross-attention (k2/v2 from enc_output) -> aw2,
  - attn2 -> LN2 -> out2, FFN -> LN3 -> out3 rows.
No cross-core communication; host assembles/transposes the per-core outputs.

Attention is computed in a transposed [k, q] layout so that:
  - aw tiles are written to DRAM with contiguous free dims (host untransposes),
  - av needs no on-chip transposes (v with an appended ones-column makes the
    softmax denominator ride along the same PE accumulation).
"""

import sys

sys.path.insert(0, "/opt/trn_rl_repo")

import contextlib

import numpy as np
import ml_dtypes

import concourse.bass as bass
import concourse.bacc as bacc
import concourse.mybir as mybir
import concourse.tile as tile
from concourse.bass_utils import run_bass_kernel_spmd
from concourse.masks import make_identity

BF16 = mybir.dt.bfloat16
F32 = mybir.dt.float32
NPBF16 = ml_dtypes.bfloat16

B, S, D, H, FF = 2, 2048, 512, 8, 2048
DK = D // H          # 64
EPS = 1e-6
N_CORES = 8
RQ = S // 4          # 512 query rows per core
DT = D // 128        # 4 contraction subtiles over D
FFT = FF // 128      # 16 FF tiles
HP = H // 2          # 4 head pairs

Exp = mybir.ActivationFunctionType.Exp
Copy = mybir.ActivationFunctionType.Copy
Ident = mybir.ActivationFunctionType.Identity
Relu = mybir.ActivationFunctionType.Relu
Sqrt = mybir.ActivationFunctionType.Sqrt
SUB = mybir.AluOpType.subtract
MULT = mybir.AluOpType.mult


def build(s=S, rq=RQ):
    kt, qt = s // 128, rq // 128
    nc = bacc.Bacc("TRN2", target_bir_lowering=False, debug=False,
                   num_devices=N_CORES)

    def din(name, shape, dt=BF16):
        return nc.dram_tensor(name, shape, dt, kind="ExternalInput")

    def dout(name, shape, dt=BF16):
        return nc.dram_tensor(name, shape, dt, kind="ExternalOutput")

    xT = din("xT", [D, s])              # x[b].T
    xqT = din("xqT", [D, rq])           # x[b, rows].T
    xres = din("xres", [rq, D], F32)    # x[b, rows] + bo1
    encT = din("encT", [D, s])          # enc[b].T
    maskTm = din("maskTm", [s, rq])     # exp(-1e9 * mask[rows, :]).T
    wq1r = din("wq1r", [128, DT, D]);  wk1r = din("wk1r", [128, DT, D])
    wv1r = din("wv1r", [128, DT, D]);  wo1r = din("wo1r", [64, H, D])
    wq2r = din("wq2r", [128, DT, D]);  wk2r = din("wk2r", [128, DT, D])
    wv2r = din("wv2r", [128, DT, D]);  wo2r = din("wo2r", [64, H, D])
    wff1r = din("wff1r", [128, DT, FF])
    wff2r = din("wff2r", [128, FFT, D])
    bq1p = din("bq1p", [128, HP], F32); bk1p = din("bk1p", [128, HP], F32)
    bq2p = din("bq2p", [128, HP], F32); bk2p = din("bk2p", [128, HP], F32)
    bff1c = din("bff1c", [128, FFT], F32)
    vecs = {}
    for nm in ["bv1", "bv2", "bo2", "bff2", "g1", "be1", "g2", "be2", "g3", "be3"]:
        vecs[nm] = din(nm, [D], F32)

    aw1T = dout("aw1T", [H, s, rq])
    aw2T = dout("aw2T", [H, s, rq])
    out3p = dout("out3p", [rq, D], F32)

    with tile.TileContext(nc) as tc, contextlib.ExitStack() as ctx:
        consts = ctx.enter_context(tc.tile_pool(name="consts", bufs=1))
        srcT_p = ctx.enter_context(tc.tile_pool(name="srcT", bufs=1))
        kT_p = ctx.enter_context(tc.tile_pool(name="kT", bufs=1))
        vaug_p = ctx.enter_context(tc.tile_pool(name="vaug", bufs=1))
        qT_p = ctx.enter_context(tc.tile_pool(name="qT", bufs=2))
        wproj_p = ctx.enter_context(tc.tile_pool(name="wproj", bufs=1))
        exp_p = ctx.enter_context(tc.tile_pool(name="expp", bufs=18))
        avn_p = ctx.enter_context(tc.tile_pool(name="avn", bufs=9))
        outs_p = ctx.enter_context(tc.tile_pool(name="outs", bufs=1))
        h1_p = ctx.enter_context(tc.tile_pool(name="h1", bufs=18))
        work_p = ctx.enter_context(tc.tile_pool(name="work", bufs=2))
        ps_l = ctx.enter_context(tc.tile_pool(name="ps_l", bufs=2, space="PSUM"))
        ps_av = ctx.enter_context(tc.tile_pool(name="ps_av", bufs=2, space="PSUM"))
        ps_m = ctx.enter_context(tc.tile_pool(name="ps_m", bufs=2, space="PSUM"))
        ps_big = ctx.enter_context(tc.tile_pool(name="ps_big", bufs=2, space="PSUM"))

        # ---- constants ----
        ones_t = consts.tile([128, 128], BF16)
        nc.vector.memset(ones_t, 1.0)
        eps_t = consts.tile([128, 1], F32)
        nc.vector.memset(eps_t, EPS)
        ident = consts.tile([128, 128], F32)
        make_identity(nc, ident)
        bcast_p = ctx.enter_context(tc.tile_pool(name="bcast", bufs=1))

        def load_bcast(nm, tag):
            t = bcast_p.tile([128, D], F32, tag=tag, name=f"bc_{nm}")
            ap = bass.AP(tensor=vecs[nm].ap().tensor, offset=0, ap=[[0, 128], [1, D]])
            nc.sync.dma_start(out=t, in_=ap)
            return t
        bcols = {}
        for nm, hnd in [("bq1p", bq1p), ("bk1p", bk1p), ("bq2p", bq2p),
                        ("bk2p", bk2p), ("bff1c", bff1c)]:
            t = consts.tile(list(hnd.shape), F32, tag=f"col_{nm}")
            nc.sync.dma_start(out=t, in_=hnd[:, :])
            bcols[nm] = t
        maskm = consts.tile([128, kt, rq], BF16)
        nc.sync.dma_start(
            out=maskm, in_=maskTm.ap().rearrange("(k p) q -> p k q", p=128))
        xqT_sb = consts.tile([128, DT, rq], BF16)
        nc.sync.dma_start(
            out=xqT_sb, in_=xqT.ap().rearrange("(t p) q -> p t q", p=128))

        def load_srcT(src):
            t = srcT_p.tile([128, DT, s], BF16, tag="srcT")
            nc.sync.dma_start(
                out=t, in_=src.ap().rearrange("(t p) q -> p t q", p=128))
            return t

        def load_w(hnd, shape, tag):
            t = wproj_p.tile(shape, BF16, tag=tag)
            nc.sync.dma_start(out=t, in_=hnd[:, :, :])
            return t

        def proj_qk(w_sb, rhs_sb, ncols, bias_col, pool, tag):
            """-> [128 (2 heads x dk), HP, ncols]: w.T @ rhs, per head pair."""
            out_t = pool.tile([128, HP, ncols], BF16, tag=tag)
            nchunk = max(1, ncols // 512)
            w = ncols // nchunk
            for p in range(HP):
                for n in range(nchunk):
                    ps = ps_l.tile([128, 512], F32, tag="ps_l", name="ps_qk")[:, :w]
                    for dt_i in range(DT):
                        nc.tensor.matmul(
                            ps, lhsT=w_sb[:, dt_i, p * 128:(p + 1) * 128],
                            rhs=rhs_sb[:, dt_i, n * w:(n + 1) * w],
                            start=(dt_i == 0), stop=(dt_i == DT - 1))
                    nc.scalar.activation(
                        out=out_t[:, p, n * w:(n + 1) * w], in_=ps,
                        func=Ident, bias=bias_col[:, p:p + 1])
            return out_t

        def proj_v(w_sb, rhs_sb, bias_bc, tag):
            """-> vaug [128, H, kt, 65] (v rows natural + ones col)."""
            va = vaug_p.tile([128, H, kt, 65], BF16, tag=tag)
            nc.vector.memset(va, 1.0)
            for k_i in range(kt):
                ps = ps_big.tile([128, 512], F32, tag="ps_big")
                for dt_i in range(DT):
                    nc.tensor.matmul(
                        ps, lhsT=rhs_sb[:, dt_i, k_i * 128:(k_i + 1) * 128],
                        rhs=w_sb[:, dt_i, :],
                        start=(dt_i == 0), stop=(dt_i == DT - 1))
                for h in range(H):
                    nc.vector.tensor_add(
                        va[:, h, k_i, 0:64], ps[:, h * 64:(h + 1) * 64],
                        bias_bc[:, h * 64:(h + 1) * 64])
            return va

        def mha(kT_sb, va_sb, qT_sb, awT_dram, masked):
            """per-head transposed attention; returns avn tiles [64, rq]."""
            avns = []
            for h in range(H):
                p, base = h // 2, (h % 2) * 64
                pav = ps_av.tile([65, 512], F32, tag="ps_av", name="pav")[:, :rq]
                exps = []
                for i in range(kt):
                    pl = ps_l.tile([128, 512], F32, tag="ps_l", name="pl")[:, :rq]
                    nc.tensor.matmul(
                        pl, lhsT=kT_sb[base:base + 64, p, i * 128:(i + 1) * 128],
                        rhs=qT_sb[base:base + 64, p, :],
                        start=True, stop=True)
                    e = exp_p.tile([128, rq], BF16, tag="exp")
                    nc.scalar.activation(out=e, in_=pl, func=Exp, scale=0.125)
                    if masked:
                        nc.vector.tensor_mul(e, e, maskm[:, i, :])
                    nc.tensor.matmul(
                        pav, lhsT=va_sb[:, h, i, :], rhs=e,
                        start=(i == 0), stop=(i == kt - 1))
                    exps.append(e)
                # reciprocal of row-sums (psum row 64), broadcast to [128, rq]
                rrow = work_p.tile([65, rq], F32, tag="rrow")
                nc.vector.reciprocal(out=rrow[64:65, :], in_=pav[64:65, :])
                rrow_b = work_p.tile([65, rq], BF16, tag="rrow_b")
                nc.vector.tensor_copy(rrow_b[64:65, :], rrow[64:65, :])
                pbc = ps_m.tile([128, 512], F32, tag="ps_m", name="pbc")[:, :rq]
                nc.tensor.matmul(pbc, lhsT=ones_t[64:65, :],
                                 rhs=rrow_b[64:65, :], start=True, stop=True)
                rbc = work_p.tile([128, rq], BF16, tag="rbc")
                nc.scalar.copy(rbc, pbc)
                avn = avn_p.tile([64, rq], BF16, tag="avn")
                nc.vector.tensor_mul(avn, pav[0:64, :], rbc[0:64, :])
                avns.append(avn)
                for i in range(kt):
                    nc.vector.tensor_mul(exps[i], exps[i], rbc)
                    nc.sync.dma_start(
                        out=awT_dram[h, i * 128:(i + 1) * 128, :], in_=exps[i])
            return avns

        def layer_norm(t, g_bc, be_bc):
            stats = work_p.tile([128, 6], F32, tag="stats")
            nc.vector.bn_stats(out=stats, in_=t)
            mv = work_p.tile([128, 2], F32, tag="mv")
            nc.vector.bn_aggr(out=mv, in_=stats)
            std = work_p.tile([128, 1], F32, tag="std")
            nc.scalar.activation(out=std, in_=mv[:, 1:2], func=Sqrt, bias=eps_t)
            rstd = work_p.tile([128, 1], F32, tag="rstd")
            nc.vector.reciprocal(out=rstd, in_=std)
            nc.vector.tensor_scalar(t, t, mv[:, 0:1], rstd, op0=SUB, op1=MULT)
            nc.vector.tensor_mul(t, t, g_bc)
            nc.vector.tensor_add(t, t, be_bc)

        def wo_ln(avns, wo_sb, res_of_qi, extra_bc, g_bc, be_bc, tag):
            """attn = concat(avn) @ wo ; t = attn + res (+extra) ; LN."""
            o = outs_p.tile([128, qt, D], F32, tag=tag)
            for q_i in range(qt):
                pa = ps_big.tile([128, 512], F32, tag="ps_big", name="pa")[:, :D]
                for h in range(H):
                    nc.tensor.matmul(
                        pa, lhsT=avns[h][:, q_i * 128:(q_i + 1) * 128],
                        rhs=wo_sb[:, h, :], start=(h == 0), stop=(h == H - 1))
                t = o[:, q_i, :]
                nc.vector.tensor_add(t, pa, res_of_qi(q_i))
                if extra_bc is not None:
                    nc.vector.tensor_add(t, t, extra_bc)
                layer_norm(t, g_bc, be_bc)
            return o

        def transpose_out(o_sb, tag):
            """[128, qt, D] f32 -> [128, DT, rq] bf16 via PE transpose."""
            oT = qT_p.tile([128, DT, rq], BF16, tag=tag)
            for q_i in range(qt):
                for dt_i in range(DT):
                    pt = ps_m.tile([128, 512], F32, tag="ps_m", name="pt")[:, :128]
                    nc.tensor.transpose(
                        pt, o_sb[:, q_i, dt_i * 128:(dt_i + 1) * 128], ident)
                    nc.scalar.copy(oT[:, dt_i, q_i * 128:(q_i + 1) * 128], pt)
            return oT

        def xres_tile(q_i):
            t = work_p.tile([128, D], F32, tag="res_in")
            nc.sync.dma_start(
                out=t, in_=xres[q_i * 128:(q_i + 1) * 128, :])
            return t

        # ================= stage A: x projections =================
        xT_sb = load_srcT(xT)
        wq1_sb = load_w(wq1r, [128, DT, D], "wq")
        wk1_sb = load_w(wk1r, [128, DT, D], "wk")
        wv1_sb = load_w(wv1r, [128, DT, D], "wv")
        wo1_sb = load_w(wo1r, [64, H, D], "wo")
        k1T = proj_qk(wk1_sb, xT_sb, s, bcols["bk1p"], kT_p, "kT")
        q1T = proj_qk(wq1_sb, xqT_sb, rq, bcols["bq1p"], qT_p, "qTp")
        bv1_bc = load_bcast("bv1", "bcv")
        v1a = proj_v(wv1_sb, xT_sb, bv1_bc, "vaug")
        # ================= mha1 + LN1 =================
        avns1 = mha(k1T, v1a, q1T, aw1T, masked=True)
        out1 = wo_ln(avns1, wo1_sb, xres_tile, None,
                     load_bcast("g1", "bcg"), load_bcast("be1", "bcbe"), "out1")
        out1T = transpose_out(out1, "outT")
        # ================= stage D: enc projections =================
        encT_sb = load_srcT(encT)
        wq2_sb = load_w(wq2r, [128, DT, D], "wq")
        wk2_sb = load_w(wk2r, [128, DT, D], "wk")
        wv2_sb = load_w(wv2r, [128, DT, D], "wv")
        wo2_sb = load_w(wo2r, [64, H, D], "wo")
        k2T = proj_qk(wk2_sb, encT_sb, s, bcols["bk2p"], kT_p, "kT")
        q2T = proj_qk(wq2_sb, out1T, rq, bcols["bq2p"], qT_p, "qTp")
        bv2_bc = load_bcast("bv2", "bcv")
        v2a = proj_v(wv2_sb, encT_sb, bv2_bc, "vaug")
        # ================= mha2 + LN2 =================
        avns2 = mha(k2T, v2a, q2T, aw2T, masked=False)
        out2 = wo_ln(avns2, wo2_sb, lambda q_i: out1[:, q_i, :],
                     load_bcast("bo2", "bcx"),
                     load_bcast("g2", "bcg"), load_bcast("be2", "bcbe"), "out2")
        out2T = transpose_out(out2, "outT")
        # ================= FFN + LN3 =================
        wff1_sb = srcT_p.tile([128, DT, FF], BF16, tag="srcT")
        nc.sync.dma_start(out=wff1_sb, in_=wff1r[:, :, :])
        wff2_sb = kT_p.tile([128, FFT, D], BF16, tag="kT")
        nc.sync.dma_start(out=wff2_sb, in_=wff2r[:, :, :])
        h1s = []
        for f_i in range(FFT):
            pf = ps_big.tile([128, 512], F32, tag="ps_big", name="pf")[:, :rq]
            for dt_i in range(DT):
                nc.tensor.matmul(
                    pf, lhsT=wff1_sb[:, dt_i, f_i * 128:(f_i + 1) * 128],
                    rhs=out2T[:, dt_i, :],
                    start=(dt_i == 0), stop=(dt_i == DT - 1))
            h1 = h1_p.tile([128, rq], BF16, tag="h1")
            nc.scalar.activation(out=h1, in_=pf, func=Relu,
                                 bias=bcols["bff1c"][:, f_i:f_i + 1])
            h1s.append(h1)
        bff2_bc = load_bcast("bff2", "bcx")
        g3_bc = load_bcast("g3", "bcg")
        be3_bc = load_bcast("be3", "bcbe")
        for q_i in range(qt):
            po = ps_big.tile([128, 512], F32, tag="ps_big", name="po")[:, :D]
            for f_i in range(FFT):
                nc.tensor.matmul(
                    po, lhsT=h1s[f_i][:, q_i * 128:(q_i + 1) * 128],
                    rhs=wff2_sb[:, f_i, :],
                    start=(f_i == 0), stop=(f_i == FFT - 1))
            t = work_p.tile([128, D], F32, tag="t3")
            nc.vector.tensor_add(t, po, out2[:, q_i, :])
            nc.vector.tensor_add(t, t, bff2_bc)
            layer_norm(t, g3_bc, be3_bc)
            nc.sync.dma_start(out=out3p[q_i * 128:(q_i + 1) * 128, :], in_=t)

    nc.compile()
    return nc


def _resh_w(w, np_dt=NPBF16):
    # [D, D2] -> [128, D//128, D2]: partition = position within 128-subtile
    d, d2 = w.shape
    return np.ascontiguousarray(
        w.reshape(d // 128, 128, d2).transpose(1, 0, 2)).astype(np_dt)


def _resh_wo(w):
    # [D, D] -> [64, H, D]
    return np.ascontiguousarray(
        w.reshape(H, 64, D).transpose(1, 0, 2)).astype(NPBF16)


def _pair_col(b):
    # [D] -> [128, HP]
    return np.ascontiguousarray(b.reshape(HP, 128).T).astype(np.float32)


def make_in_maps(inputs, s=S, rq=RQ):
    f32 = lambda a: np.asarray(a, np.float32)
    bf = lambda a: np.ascontiguousarray(a).astype(NPBF16)
    x = f32(inputs["x"]); enc = f32(inputs["enc_output"])
    mask = f32(inputs["look_ahead_mask"])[0, 0]
    base = dict(
        wq1r=_resh_w(f32(inputs["wq1"])), wk1r=_resh_w(f32(inputs["wk1"])),
        wv1r=_resh_w(f32(inputs["wv1"])), wo1r=_resh_wo(f32(inputs["wo1"])),
        wq2r=_resh_w(f32(inputs["wq2"])), wk2r=_resh_w(f32(inputs["wk2"])),
        wv2r=_resh_w(f32(inputs["wv2"])), wo2r=_resh_wo(f32(inputs["wo2"])),
        wff1r=_resh_w(f32(inputs["w_ff1"])),
        wff2r=np.ascontiguousarray(
            f32(inputs["w_ff2"]).reshape(FFT, 128, D).transpose(1, 0, 2)
        ).astype(NPBF16),
        bq1p=_pair_col(f32(inputs["bq1"])), bk1p=_pair_col(f32(inputs["bk1"])),
        bq2p=_pair_col(f32(inputs["bq2"])), bk2p=_pair_col(f32(inputs["bk2"])),
        bff1c=np.ascontiguousarray(
            f32(inputs["b_ff1"]).reshape(FFT, 128).T).astype(np.float32),
        bv1=f32(inputs["bv1"]), bv2=f32(inputs["bv2"]),
        bo2=f32(inputs["bo2"]), bff2=f32(inputs["b_ff2"]),
        g1=f32(inputs["g1"]), be1=f32(inputs["be1"]),
        g2=f32(inputs["g2"]), be2=f32(inputs["be2"]),
        g3=f32(inputs["g3"]), be3=f32(inputs["be3"]),
    )
    in_maps = []
    for c in range(N_CORES):
        b, r = c // 4, c % 4
        rows = slice(r * rq, (r + 1) * rq)
        m = dict(base)
        m["xT"] = bf(x[b].T)
        m["xqT"] = bf(x[b, rows].T)
        m["xres"] = np.ascontiguousarray(
            x[b, rows] + f32(inputs["bo1"])[None, :]).astype(np.float32)
        m["encT"] = bf(enc[b].T)
        mm = np.exp(np.float64(-1e9) * np.float64(mask[rows, :])).astype(np.float32)
        m["maskTm"] = np.ascontiguousarray(mm.T).astype(NPBF16)
        in_maps.append(m)
    return in_maps


_NC_CACHE = {}


def run_cores(inputs, trace=False):
    if "nc" not in _NC_CACHE:
        _NC_CACHE["nc"] = build()
    nc = _NC_CACHE["nc"]
    in_maps = make_in_maps(inputs)
    return run_bass_kernel_spmd(nc, in_maps, core_ids=list(range(N_CORES)),
                                trace=trace)


def assemble(results):
    out3 = np.empty((B, S, D), np.float32)
    aw1 = np.empty((B, H, S, S), np.float32)
    aw2 = np.empty((B, H, S, S), np.float32)
    for c in range(N_CORES):
        b, r = c // 4, c % 4
        rows = slice(r * RQ, (r + 1) * RQ)
        rc = results[c]
        aw1[b, :, rows, :] = rc["aw1T"].astype(np.float32).transpose(0, 2, 1)
        aw2[b, :, rows, :] = rc["aw2T"].astype(np.float32).transpose(0, 2, 1)
        out3[b, rows, :] = rc["out3p"]
    return out3, aw1, aw2


def kernel(**inputs):
    res = run_cores(inputs)
    return assemble(res.results)


# revision 11
# speedup vs baseline: 1.1509x; 1.1509x over previous
"""Trainium2 Bass kernel for a transformer decoder layer (self-attn + cross-attn + FFN).

Sharding: 8 cores = (batch b in {0,1}) x (query-row quarter r in {0..3}).
Each core computes, for its 512 query rows of its batch:
  - q1/k1/v1 projections (k1/v1 over the full 2048 rows of its batch),
  - masked softmax attention weights aw1 (written transposed, bf16),
  - attn1 -> LN1 -> out1, cross-attention (k2/v2 from enc_output) -> aw2,
  - attn2 -> LN2 -> out2, FFN -> LN3 -> out3 rows.
No cross-core communication; host assembles/transposes the per-core outputs.

Attention is computed in a transposed [k, q] layout so that:
  - aw tiles are written to DRAM with contiguous free dims (host untransposes),
  - av needs no on-chip transposes (v with an appended ones-column makes the
    softmax denominator ride along the same PE accumulation).
"""

import sys

sys.path.insert(0, "/opt/trn_rl_repo")

import contextlib

import numpy as np
import ml_dtypes

import concourse.bass as bass
import concourse.bacc as bacc
import concourse.mybir as mybir
import concourse.tile as tile
from concourse.bass_utils import run_bass_kernel_spmd
from concourse.masks import make_identity

BF16 = mybir.dt.bfloat16
F32 = mybir.dt.float32
NPBF16 = ml_dtypes.bfloat16

B, S, D, H, FF = 2, 2048, 512, 8, 2048
DK = D // H          # 64
EPS = 1e-6
N_CORES = 8
RQ = S // 4          # 512 query rows per core
DT = D // 128        # 4 contraction subtiles over D
FFT = FF // 128      # 16 FF tiles
HP = H // 2          # 4 head pairs

Exp = mybir.ActivationFunctionType.Exp
Copy = mybir.ActivationFunctionType.Copy
Ident = mybir.ActivationFunctionType.Identity
Relu = mybir.ActivationFunctionType.Relu
Sqrt = mybir.ActivationFunctionType.Sqrt
SUB = mybir.AluOpType.subtract
MULT = mybir.AluOpType.mult


def build(s=S, rq=RQ):
    kt, qt = s // 128, rq // 128
    nc = bacc.Bacc("TRN2", target_bir_lowering=False, debug=False,
                   num_devices=N_CORES)

    def din(name, shape, dt=BF16):
        return nc.dram_tensor(name, shape, dt, kind="ExternalInput")

    def dout(name, shape, dt=BF16):
        return nc.dram_tensor(name, shape, dt, kind="ExternalOutput")

    xT = din("xT", [D, s])              # x[b].T
    xqT = din("xqT", [D, rq])           # x[b, rows].T
    xres = din("xres", [rq, D], F32)    # x[b, rows] + bo1
    encT = din("encT", [D, s])          # enc[b].T
    maskTm = din("maskTm", [s, rq])     # exp(-1e9 * mask[rows, :]).T
    wq1r = din("wq1r", [128, DT, D]);  wk1r = din("wk1r", [128, DT, D])
    wv1r = din("wv1r", [128, DT, D]);  wo1r = din("wo1r", [64, H, D])
    wq2r = din("wq2r", [128, DT, D]);  wk2r = din("wk2r", [128, DT, D])
    wv2r = din("wv2r", [128, DT, D]);  wo2r = din("wo2r", [64, H, D])
    wff1r = din("wff1r", [128, DT, FF])
    wff2r = din("wff2r", [128, FFT, D])
    bq1p = din("bq1p", [128, HP], F32); bk1p = din("bk1p", [128, HP], F32)
    bq2p = din("bq2p", [128, HP], F32); bk2p = din("bk2p", [128, HP], F32)
    bff1c = din("bff1c", [128, FFT], F32)
    vecs = {}
    for nm in ["bv1", "bv2", "bo2", "bff2", "g1", "be1", "g2", "be2", "g3", "be3"]:
        vecs[nm] = din(nm, [D], F32)

    aw1T = dout("aw1T", [H, s, rq])
    aw2T = dout("aw2T", [H, s, rq])
    out3p = dout("out3p", [rq, D], F32)

    with tile.TileContext(nc) as tc, contextlib.ExitStack() as ctx:
        consts = ctx.enter_context(tc.tile_pool(name="consts", bufs=1))
        srcT_p = ctx.enter_context(tc.tile_pool(name="srcT", bufs=1))
        kT_p = ctx.enter_context(tc.tile_pool(name="kT", bufs=1))
        vaug_p = ctx.enter_context(tc.tile_pool(name="vaug", bufs=1))
        qT_p = ctx.enter_context(tc.tile_pool(name="qT", bufs=2))
        wproj_p = ctx.enter_context(tc.tile_pool(name="wproj", bufs=1))
        exp_p = ctx.enter_context(tc.tile_pool(name="expp", bufs=32))
        avn_p = ctx.enter_context(tc.tile_pool(name="avn", bufs=9))
        outs_p = ctx.enter_context(tc.tile_pool(name="outs", bufs=1))
        h1_p = ctx.enter_context(tc.tile_pool(name="h1", bufs=17))
        work_p = ctx.enter_context(tc.tile_pool(name="work", bufs=2))
        ps_l = ctx.enter_context(tc.tile_pool(name="ps_l", bufs=3, space="PSUM"))
        ps_av = ctx.enter_context(tc.tile_pool(name="ps_av", bufs=2, space="PSUM"))
        ps_m = ctx.enter_context(tc.tile_pool(name="ps_m", bufs=1, space="PSUM"))
        ps_big = ctx.enter_context(tc.tile_pool(name="ps_big", bufs=2, space="PSUM"))

        # ---- constants ----
        ones_t = consts.tile([128, 128], BF16)
        nc.vector.memset(ones_t, 1.0)
        eps_t = consts.tile([128, 1], F32)
        nc.vector.memset(eps_t, EPS)
        ident = consts.tile([128, 128], F32)
        make_identity(nc, ident)
        bcast_p = ctx.enter_context(tc.tile_pool(name="bcast", bufs=1))

        def load_bcast(nm, tag):
            t = bcast_p.tile([128, D], F32, tag=tag, name=f"bc_{nm}")
            ap = bass.AP(tensor=vecs[nm].ap().tensor, offset=0, ap=[[0, 128], [1, D]])
            nc.sync.dma_start(out=t, in_=ap)
            return t
        bcols = {}
        for nm, hnd in [("bq1p", bq1p), ("bk1p", bk1p), ("bq2p", bq2p),
                        ("bk2p", bk2p), ("bff1c", bff1c)]:
            t = consts.tile(list(hnd.shape), F32, tag=f"col_{nm}")
            nc.sync.dma_start(out=t, in_=hnd[:, :])
            bcols[nm] = t
        maskm = consts.tile([128, kt, rq], BF16)
        nc.sync.dma_start(
            out=maskm, in_=maskTm.ap().rearrange("(k p) q -> p k q", p=128))
        xqT_sb = consts.tile([128, DT, rq], BF16)
        nc.sync.dma_start(
            out=xqT_sb, in_=xqT.ap().rearrange("(t p) q -> p t q", p=128))

        def load_srcT(src):
            t = srcT_p.tile([128, DT, s], BF16, tag="srcT")
            nc.sync.dma_start(
                out=t, in_=src.ap().rearrange("(t p) q -> p t q", p=128))
            return t

        def load_w(hnd, shape, tag):
            t = wproj_p.tile(shape, BF16, tag=tag)
            nc.sync.dma_start(out=t, in_=hnd[:, :, :])
            return t

        def proj_qk(w_sb, rhs_sb, ncols, bias_col, pool, tag):
            """-> [128 (2 heads x dk), HP, ncols]: w.T @ rhs, per head pair."""
            out_t = pool.tile([128, HP, ncols], BF16, tag=tag)
            nchunk = max(1, ncols // 512)
            w = ncols // nchunk
            for p in range(HP):
                for n in range(nchunk):
                    ps = ps_l.tile([128, 512], F32, tag="ps_l", name="ps_qk")[:, :w]
                    for dt_i in range(DT):
                        nc.tensor.matmul(
                            ps, lhsT=w_sb[:, dt_i, p * 128:(p + 1) * 128],
                            rhs=rhs_sb[:, dt_i, n * w:(n + 1) * w],
                            start=(dt_i == 0), stop=(dt_i == DT - 1))
                    nc.scalar.activation(
                        out=out_t[:, p, n * w:(n + 1) * w], in_=ps,
                        func=Ident, bias=bias_col[:, p:p + 1])
            return out_t

        def proj_v(w_sb, rhs_sb, bias_bc, tag):
            """-> vaug [128, H, kt, 65] (v rows natural + ones col)."""
            va = vaug_p.tile([128, H, kt, 65], BF16, tag=tag)
            nc.vector.memset(va[:, :, :, 64:65], 1.0)
            for k_i in range(kt):
                ps = ps_big.tile([128, 512], F32, tag="ps_big")
                for dt_i in range(DT):
                    nc.tensor.matmul(
                        ps, lhsT=rhs_sb[:, dt_i, k_i * 128:(k_i + 1) * 128],
                        rhs=w_sb[:, dt_i, :],
                        start=(dt_i == 0), stop=(dt_i == DT - 1))
                nc.vector.tensor_add(
                    va[:, :, k_i, 0:64],
                    ps.rearrange("p (h d) -> p h d", h=H),
                    bias_bc.rearrange("p (h d) -> p h d", h=H))
            return va

        def mha(kT_sb, va_sb, qT_sb, awT_dram, masked):
            """head-pair interleaved transposed attention; returns avn tiles."""
            avns = [None] * H
            for p in range(HP):
                pair = ((2 * p, 0), (2 * p + 1, 64))
                pavs = {}
                exps = {2 * p: [], 2 * p + 1: []}
                for h, base in pair:
                    pavs[h] = ps_av.tile([65, 512], F32, tag="ps_av",
                                         name=f"pav{h}")[:, :rq]
                for i in range(kt):
                    pls = {}
                    # both logits matmuls adjacent: disjoint PE row groups
                    for h, base in pair:
                        pl = ps_l.tile([128, 512], F32, tag="ps_l",
                                       name=f"pl{h}")[:, :rq]
                        nc.tensor.matmul(
                            pl, lhsT=kT_sb[base:base + 64, p, i * 128:(i + 1) * 128],
                            rhs=qT_sb[base:base + 64, p, :],
                            start=True, stop=True)
                        pls[h] = pl
                    for h, base in pair:
                        e = exp_p.tile([128, rq], BF16, tag="exp", name=f"e{h}")
                        nc.scalar.activation(out=e, in_=pls[h], func=Exp, scale=0.125)
                        if masked:
                            nc.vector.tensor_mul(e, e, maskm[:, i, :])
                        nc.tensor.matmul(
                            pavs[h], lhsT=va_sb[:, h, i, :], rhs=e,
                            start=(i == 0), stop=(i == kt - 1))
                        exps[h].append(e)
                for h, base in pair:
                    pav = pavs[h]
                    # fast reciprocal of row-sums, broadcast to [128, rq]
                    rrow = work_p.tile([65, rq], F32, tag="rrow")
                    nc.vector.reciprocal(out=rrow[64:65, :], in_=pav[64:65, :])
                    rrow_b = work_p.tile([65, rq], BF16, tag="rrow_b")
                    nc.vector.tensor_copy(rrow_b[64:65, :], rrow[64:65, :])
                    pbc = ps_m.tile([128, 512], F32, tag="ps_m", name="pbc")[:, :rq]
                    nc.tensor.matmul(pbc, lhsT=ones_t[64:65, :],
                                     rhs=rrow_b[64:65, :], start=True, stop=True)
                    rbc = work_p.tile([128, rq], BF16, tag="rbc")
                    nc.scalar.copy(rbc, pbc)
                    avn = avn_p.tile([64, rq], BF16, tag="avn")
                    nc.vector.tensor_mul(avn, pav[0:64, :], rbc[0:64, :])
                    avns[h] = avn
                    for i in range(kt):
                        nc.vector.tensor_mul(exps[h][i], exps[h][i], rbc)
                        nc.sync.dma_start(
                            out=awT_dram[h, i * 128:(i + 1) * 128, :],
                            in_=exps[h][i])
            return avns

        def layer_norm(t, g_bc, be_bc):
            stats = work_p.tile([128, 6], F32, tag="stats")
            nc.vector.bn_stats(out=stats, in_=t)
            mv = work_p.tile([128, 2], F32, tag="mv")
            nc.vector.bn_aggr(out=mv, in_=stats)
            std = work_p.tile([128, 1], F32, tag="std")
            nc.scalar.activation(out=std, in_=mv[:, 1:2], func=Sqrt, bias=eps_t)
            rstd = work_p.tile([128, 1], F32, tag="rstd")
            nc.vector.reciprocal(out=rstd, in_=std)
            nc.vector.tensor_scalar(t, t, mv[:, 0:1], rstd, op0=SUB, op1=MULT)
            nc.vector.tensor_mul(t, t, g_bc)
            nc.vector.tensor_add(t, t, be_bc)

        def wo_ln(avns, wo_sb, res_of_qi, extra_bc, g_bc, be_bc, tag):
            """attn = concat(avn) @ wo ; t = attn + res (+extra) ; LN."""
            o = outs_p.tile([128, qt, D], F32, tag=tag)
            for q_i in range(qt):
                pa = ps_big.tile([128, 512], F32, tag="ps_big", name="pa")[:, :D]
                for h in range(H):
                    nc.tensor.matmul(
                        pa, lhsT=avns[h][:, q_i * 128:(q_i + 1) * 128],
                        rhs=wo_sb[:, h, :], start=(h == 0), stop=(h == H - 1))
                t = o[:, q_i, :]
                nc.vector.tensor_add(t, pa, res_of_qi(q_i))
                if extra_bc is not None:
                    nc.vector.tensor_add(t, t, extra_bc)
                layer_norm(t, g_bc, be_bc)
            return o

        def transpose_out(o_sb, tag):
            """[128, qt, D] f32 -> [128, DT, rq] bf16 via PE transpose."""
            oT = qT_p.tile([128, DT, rq], BF16, tag=tag)
            for q_i in range(qt):
                for dt_i in range(DT):
                    pt = ps_m.tile([128, 512], F32, tag="ps_m", name="pt")[:, :128]
                    nc.tensor.transpose(
                        pt, o_sb[:, q_i, dt_i * 128:(dt_i + 1) * 128], ident)
                    nc.scalar.copy(oT[:, dt_i, q_i * 128:(q_i + 1) * 128], pt)
            return oT

        def xres_tile(q_i):
            t = work_p.tile([128, D], F32, tag="res_in")
            nc.sync.dma_start(
                out=t, in_=xres[q_i * 128:(q_i + 1) * 128, :])
            return t

        # ================= stage A: x projections =================
        xT_sb = load_srcT(xT)
        wq1_sb = load_w(wq1r, [128, DT, D], "wq")
        wk1_sb = load_w(wk1r, [128, DT, D], "wk")
        wv1_sb = load_w(wv1r, [128, DT, D], "wv")
        wo1_sb = load_w(wo1r, [64, H, D], "wo")
        k1T = proj_qk(wk1_sb, xT_sb, s, bcols["bk1p"], kT_p, "kT")
        q1T = proj_qk(wq1_sb, xqT_sb, rq, bcols["bq1p"], qT_p, "qTp")
        bv1_bc = load_bcast("bv1", "bcv")
        v1a = proj_v(wv1_sb, xT_sb, bv1_bc, "vaug")
        # ================= mha1 + LN1 =================
        avns1 = mha(k1T, v1a, q1T, aw1T, masked=True)
        out1 = wo_ln(avns1, wo1_sb, xres_tile, None,
                     load_bcast("g1", "bcg"), load_bcast("be1", "bcbe"), "out1")
        out1T = transpose_out(out1, "outT")
        # ================= stage D: enc projections =================
        encT_sb = load_srcT(encT)
        wq2_sb = load_w(wq2r, [128, DT, D], "wq")
        wk2_sb = load_w(wk2r, [128, DT, D], "wk")
        wv2_sb = load_w(wv2r, [128, DT, D], "wv")
        wo2_sb = load_w(wo2r, [64, H, D], "wo")
        k2T = proj_qk(wk2_sb, encT_sb, s, bcols["bk2p"], kT_p, "kT")
        q2T = proj_qk(wq2_sb, out1T, rq, bcols["bq2p"], qT_p, "qTp")
        bv2_bc = load_bcast("bv2", "bcv")
        v2a = proj_v(wv2_sb, encT_sb, bv2_bc, "vaug")
        # ================= mha2 + LN2 =================
        avns2 = mha(k2T, v2a, q2T, aw2T, masked=False)
        out2 = wo_ln(avns2, wo2_sb, lambda q_i: out1[:, q_i, :],
                     load_bcast("bo2", "bcx"),
                     load_bcast("g2", "bcg"), load_bcast("be2", "bcbe"), "out2")
        out2T = transpose_out(out2, "outT")
        # ================= FFN + LN3 =================
        wff1_sb = srcT_p.tile([128, DT, FF], BF16, tag="srcT")
        nc.sync.dma_start(out=wff1_sb, in_=wff1r[:, :, :])
        wff2_sb = kT_p.tile([128, FFT, D], BF16, tag="kT")
        nc.sync.dma_start(out=wff2_sb, in_=wff2r[:, :, :])
        h1s = []
        for f_i in range(FFT):
            pf = ps_big.tile([128, 512], F32, tag="ps_big", name="pf")[:, :rq]
            for dt_i in range(DT):
                nc.tensor.matmul(
                    pf, lhsT=wff1_sb[:, dt_i, f_i * 128:(f_i + 1) * 128],
                    rhs=out2T[:, dt_i, :],
                    start=(dt_i == 0), stop=(dt_i == DT - 1))
            h1 = h1_p.tile([128, rq], BF16, tag="h1")
            nc.scalar.activation(out=h1, in_=pf, func=Relu,
                                 bias=bcols["bff1c"][:, f_i:f_i + 1])
            h1s.append(h1)
        bff2_bc = load_bcast("bff2", "bcx")
        g3_bc = load_bcast("g3", "bcg")
        be3_bc = load_bcast("be3", "bcbe")
        for q_i in range(qt):
            po = ps_big.tile([128, 512], F32, tag="ps_big", name="po")[:, :D]
            for f_i in range(FFT):
                nc.tensor.matmul(
                    po, lhsT=h1s[f_i][:, q_i * 128:(q_i + 1) * 128],
                    rhs=wff2_sb[:, f_i, :],
                    start=(f_i == 0), stop=(f_i == FFT - 1))
            t = work_p.tile([128, D], F32, tag="t3")
            nc.vector.tensor_add(t, po, out2[:, q_i, :])
            nc.vector.tensor_add(t, t, bff2_bc)
            layer_norm(t, g3_bc, be3_bc)
            nc.sync.dma_start(out=out3p[q_i * 128:(q_i + 1) * 128, :], in_=t)

    nc.compile()
    return nc


def _resh_w(w, np_dt=NPBF16):
    # [D, D2] -> [128, D//128, D2]: partition = position within 128-subtile
    d, d2 = w.shape
    return np.ascontiguousarray(
        w.reshape(d // 128, 128, d2).transpose(1, 0, 2)).astype(np_dt)


def _resh_wo(w):
    # [D, D] -> [64, H, D]
    return np.ascontiguousarray(
        w.reshape(H, 64, D).transpose(1, 0, 2)).astype(NPBF16)


def _pair_col(b):
    # [D] -> [128, HP]
    return np.ascontiguousarray(b.reshape(HP, 128).T).astype(np.float32)


def make_in_maps(inputs, s=S, rq=RQ):
    f32 = lambda a: np.asarray(a, np.float32)
    bf = lambda a: np.ascontiguousarray(a).astype(NPBF16)
    x = f32(inputs["x"]); enc = f32(inputs["enc_output"])
    mask = f32(inputs["look_ahead_mask"])[0, 0]
    base = dict(
        wq1r=_resh_w(f32(inputs["wq1"])), wk1r=_resh_w(f32(inputs["wk1"])),
        wv1r=_resh_w(f32(inputs["wv1"])), wo1r=_resh_wo(f32(inputs["wo1"])),
        wq2r=_resh_w(f32(inputs["wq2"])), wk2r=_resh_w(f32(inputs["wk2"])),
        wv2r=_resh_w(f32(inputs["wv2"])), wo2r=_resh_wo(f32(inputs["wo2"])),
        wff1r=_resh_w(f32(inputs["w_ff1"])),
        wff2r=np.ascontiguousarray(
            f32(inputs["w_ff2"]).reshape(FFT, 128, D).transpose(1, 0, 2)
        ).astype(NPBF16),
        bq1p=_pair_col(f32(inputs["bq1"])), bk1p=_pair_col(f32(inputs["bk1"])),
        bq2p=_pair_col(f32(inputs["bq2"])), bk2p=_pair_col(f32(inputs["bk2"])),
        bff1c=np.ascontiguousarray(
            f32(inputs["b_ff1"]).reshape(FFT, 128).T).astype(np.float32),
        bv1=f32(inputs["bv1"]), bv2=f32(inputs["bv2"]),
        bo2=f32(inputs["bo2"]), bff2=f32(inputs["b_ff2"]),
        g1=f32(inputs["g1"]), be1=f32(inputs["be1"]),
        g2=f32(inputs["g2"]), be2=f32(inputs["be2"]),
        g3=f32(inputs["g3"]), be3=f32(inputs["be3"]),
    )
    in_maps = []
    for c in range(N_CORES):
        b, r = c // 4, c % 4
        rows = slice(r * rq, (r + 1) * rq)
        m = dict(base)
        m["xT"] = bf(x[b].T)
        m["xqT"] = bf(x[b, rows].T)
        m["xres"] = np.ascontiguousarray(
            x[b, rows] + f32(inputs["bo1"])[None, :]).astype(np.float32)
        m["encT"] = bf(enc[b].T)
        mm = np.exp(np.float64(-1e9) * np.float64(mask[rows, :])).astype(np.float32)
        m["maskTm"] = np.ascontiguousarray(mm.T).astype(NPBF16)
        in_maps.append(m)
    return in_maps


_NC_CACHE = {}


def _axon_reset():
    try:
        import ctypes
        lib = ctypes.CDLL("/opt/axon/libaxon_pjrt.so")
        lib.axon_reset.restype = ctypes.c_int64
        lib.axon_reset()
    except Exception:
        pass


def run_cores(inputs, trace=False):
    if "nc" not in _NC_CACHE:
        _NC_CACHE["nc"] = build()
    nc = _NC_CACHE["nc"]
    in_maps = make_in_maps(inputs)
    try:
        return run_bass_kernel_spmd(nc, in_maps, core_ids=list(range(N_CORES)),
                                    trace=trace)
    except Exception:
        _axon_reset()
        return run_bass_kernel_spmd(nc, in_maps, core_ids=list(range(N_CORES)),
                                    trace=trace)


def assemble(results):
    out3 = np.empty((B, S, D), np.float32)
    aw1 = np.empty((B, H, S, S), np.float32)
    aw2 = np.empty((B, H, S, S), np.float32)
    for c in range(N_CORES):
        b, r = c // 4, c % 4
        rows = slice(r * RQ, (r + 1) * RQ)
        rc = results[c]
        aw1[b, :, rows, :] = rc["aw1T"].astype(np.float32).transpose(0, 2, 1)
        aw2[b, :, rows, :] = rc["aw2T"].astype(np.float32).transpose(0, 2, 1)
        out3[b, rows, :] = rc["out3p"]
    return out3, aw1, aw2


def kernel(**inputs):
    res = run_cores(inputs)
    return assemble(res.results)
